# revision 13
# baseline (speedup 1.0000x reference)
"""Conv2D (VALID, 3x3, NCHW) on 8 TRN2 NeuronCores via Bass/Tile.

Problem: x (32,128,56,56) f32, weight (256,128,3,3) f32, bias (256,) f32
         -> out (32,256,54,54) f32.

Strategy:
  - Data-parallel over batch: 4 images per core, 8 cores, no collectives.
  - Conv as implicit GEMM: for each kernel tap (kh,kw), a matmul with
    lhsT = weight[ci, co_tile] (K=Cin=128 partitions, M=128) and
    rhs  = shifted x window [Cin=128, 9 rows x 54 cols = 486], accumulating
    all 9 taps into one PSUM bank. 2 cout tiles x 6 row groups x 4 images
    = 48 accumulation groups x 9 matmuls per core.
  - Inputs cast to bf16 on host (PE runs at full rate); accumulation fp32.
  - Output copied PSUM->SBUF as bf16 (halves output DMA bytes); bias add
    and f32 upcast happen on host.
  - Graduated PE warmup (small then full-width matmuls) keeps the PE busy
    through the input-DMA window so HAM is at full clock (k=8) when the
    real stream starts, with no idle gap that would trigger a downshift.
  - Critical input DMAs split so the first taps' weights and first rows of
    x arrive first; everything else is deferred behind early matmuls so
    the SDMA round-robin doesn't steal bandwidth from the critical path.
"""

import numpy as np
import ml_dtypes

import concourse.bass as bass
import concourse.mybir as mybir
from concourse import bacc
import concourse.tile as tile
from concourse.tile import add_dep_helper
from concourse.bass_utils import run_bass_kernel_spmd

N, CIN, H, W = 32, 128, 56, 56
COUT, KH, KW = 256, 3, 3
HO, WO = H - KH + 1, W - KW + 1  # 54, 54
NCORES = 8
NPER = N // NCORES  # 4 images per core
CTILES = COUT // 128  # 2
RG = 9                # output rows per PSUM group
NG = HO // RG         # 6 row groups
NPIX = RG * WO        # 486 <= 512 (one fp32 PSUM bank)

N_WU_SMALL = 33       # small warmup matmuls (32x64)
N_WU_BIG = 7          # full-width warmup matmuls (128x486)

BF16 = mybir.dt.bfloat16
F32 = mybir.dt.float32


def build_nc() -> bass.Bass:
    nc = bacc.Bacc(None)
    x_h = nc.dram_tensor("x", [NPER, CIN, H, W], BF16, kind="ExternalInput")
    w_h = nc.dram_tensor("w", [CIN, KH * KW * COUT], BF16, kind="ExternalInput")
    o_h = nc.dram_tensor("out", [NPER, COUT, HO, WO], BF16, kind="ExternalOutput")

    with tile.TileContext(nc) as tc:
        with (
            tc.tile_pool(name="wpool", bufs=1) as wpool,
            tc.tile_pool(name="xpool", bufs=4) as xpool,
            tc.tile_pool(name="opool", bufs=4) as opool,
            tc.tile_pool(name="psum", bufs=8, space="PSUM") as psum_pool,
        ):
            # PE warmup: matmuls on memset data with no DMA deps keep the PE
            # busy during the input-DMA window so HAM un-throttles to full
            # clock by the time the real matmuls begin. Graduated: small
            # matmuls first, then full-width ones so the power step of the
            # real stream doesn't trigger a downshift.
            # memset on DVE: keeps GpSimd entirely unused so its queue/boot
            # cost drops out of the NEFF.
            wu = wpool.tile([CIN, 640], BF16)
            nc.vector.memset(wu[:], 0)
            wupt = psum_pool.tile([32, 64], F32, tag="pt")
            warmups = []
            for _ in range(N_WU_SMALL):
                warmups.append(
                    nc.tensor.matmul(wupt[:], wu[:, :32], wu[:, :64], start=True, stop=True)
                )
            wupt2 = psum_pool.tile([128, NPIX], F32, tag="pt")
            for i in range(N_WU_BIG):
                warmups.append(
                    nc.tensor.matmul(
                        wupt2[:], wu[:, :128], wu[:, 128 : 128 + NPIX],
                        start=(i == 0), stop=(i == N_WU_BIG - 1),
                    )
                )

            # Input DMAs split across the two HWDGE rings (each is FIFO):
            # weights on sync (SP) in parallel with the first x0 chunk on
            # scalar (ACT). The weight stream is split per-tap-triplet so the
            # first real matmul only waits on taps 0-2; later chunks arrive
            # while the first groups run.
            wt = wpool.tile([CIN, KH * KW * COUT], BF16)
            # Tiny ring-warmer DMAs: get both HWDGE rings fetching before the
            # real descriptors land so their first-pickup latency is paid on
            # a 4-byte transfer instead of the critical input chunks.
            nc.sync.dma_start(out=wt[0:1, 0:2], in_=w_h[0:1, 0:2])
            nc.scalar.dma_start(out=wt[0:1, 2:4], in_=w_h[0:1, 2:4])
            nc.sync.dma_start(out=wt[:, : 3 * COUT], in_=w_h[:, : 3 * COUT])
            wB = nc.sync.dma_start(
                out=wt[:, 3 * COUT : 6 * COUT], in_=w_h[:, 3 * COUT : 6 * COUT]
            )
            wC = nc.sync.dma_start(out=wt[:, 6 * COUT :], in_=w_h[:, 6 * COUT :])

            xts = []
            for n in range(NPER):
                xt = xpool.tile([CIN, H, W], BF16, tag="xt", name=f"xt{n}")
                xts.append(xt)
            # group 0 needs x rows 0-10 only
            nc.scalar.dma_start(out=xts[0][:, 0:11, :], in_=x_h[0, :, 0:11, :])
            x0b = nc.scalar.dma_start(out=xts[0][:, 11:30, :], in_=x_h[0, :, 11:30, :])
            x0c = nc.scalar.dma_start(out=xts[0][:, 30:56, :], in_=x_h[0, :, 30:56, :])
            xdmas = [None]
            for n in range(1, NPER):
                xdmas.append(nc.sync.dma_start(out=xts[n][:], in_=x_h[n]))

            # Defer non-critical input DMAs behind warmup/early matmuls so
            # the SDMA round-robin doesn't steal bandwidth from the
            # transfers the first matmuls need. wB/wC stay undeferred: the
            # sync ring is FIFO, so they stream right behind wA.
            add_dep_helper(x0b.ins, warmups[30].ins, reason="defer x0b")
            deferred = {0: [x0c], 18: [xdmas[1]], 108: [xdmas[2]], 216: [xdmas[3]]}
            mm_idx = 0

            for n in range(NPER):
                xt = xts[n]
                for c in range(CTILES):
                    for g in range(NG):
                        pt = psum_pool.tile([128, RG, WO], F32, tag="pt")
                        for t in range(KH * KW):
                            kh, kw = divmod(t, KW)
                            lhsT = wt[:, t * COUT + c * 128 : t * COUT + c * 128 + 128]
                            rhs = xt[:, g * RG + kh : g * RG + kh + RG, kw : kw + WO]
                            mm = nc.tensor.matmul(
                                pt[:], lhsT, rhs,
                                start=(t == 0), stop=(t == KH * KW - 1),
                            )
                            for dma in deferred.get(mm_idx, ()):
                                add_dep_helper(dma.ins, mm.ins, reason="defer DMA")
                            mm_idx += 1
                        ot = opool.tile([128, RG, WO], BF16, tag="ot")
                        last = n == NPER - 1 and c == CTILES - 1 and g == NG - 1
                        co = c * 128
                        # Output DMAs ride the scalar (ACT) HWDGE ring so their
                        # sem waits never head-of-line block the input ring.
                        if not last:
                            nc.vector.tensor_copy(ot[:], pt[:])
                            nc.scalar.dma_start(
                                out=o_h[n, co : co + 128, g * RG : (g + 1) * RG, :],
                                in_=ot[:],
                            )
                        else:
                            # Split the final copy/DMA so the store pipeline
                            # drains sooner after the last matmul.
                            for eng, (ra, rb) in (
                                (nc.scalar, (0, 5)),
                                (nc.sync, (5, RG)),
                            ):
                                nc.vector.tensor_copy(ot[:, ra:rb, :], pt[:, ra:rb, :])
                                eng.dma_start(
                                    out=o_h[n, co : co + 128, g * RG + ra : g * RG + rb, :],
                                    in_=ot[:, ra:rb, :],
                                )
    nc.finalize()
    return nc


_NC_CACHE = None


def _get_nc():
    global _NC_CACHE
    if _NC_CACHE is None:
        _NC_CACHE = build_nc()
    return _NC_CACHE


def _prep_in_maps(x, weight):
    bf16 = ml_dtypes.bfloat16
    # [ci, kh, kw, co] layout so lhsT slices are [ci, co_tile]
    w_t = np.ascontiguousarray(
        weight.astype(np.float32).transpose(1, 2, 3, 0).reshape(CIN, KH * KW * COUT)
    ).astype(bf16)
    in_maps = []
    for i in range(NCORES):
        xs = np.ascontiguousarray(x[i * NPER : (i + 1) * NPER]).astype(bf16)
        in_maps.append({"x": xs, "w": w_t})
    return in_maps


def run(x, weight, bias, trace=False):
    nc = _get_nc()
    in_maps = _prep_in_maps(x, weight)
    res = run_bass_kernel_spmd(nc, in_maps, core_ids=list(range(NCORES)), trace=trace)
    out = np.concatenate([r["out"] for r in res.results], axis=0).astype(np.float32)
    bias = np.asarray(bias, dtype=np.float32)
    if np.any(bias):
        out += bias[None, :, None, None]
    return out, res


def kernel(x: np.ndarray, weight: np.ndarray, bias: np.ndarray) -> np.ndarray:
    out, _ = run(x, weight, bias, trace=False)
    return out.astype(np.float32)


# revision 14
# speedup vs baseline: 1.0096x; 1.0096x over previous
"""Conv2D (VALID, 3x3, NCHW) on 8 TRN2 NeuronCores via Bass/Tile.

Problem: x (32,128,56,56) f32, weight (256,128,3,3) f32, bias (256,) f32
         -> out (32,256,54,54) f32.

Strategy:
  - Data-parallel over batch: 4 images per core, 8 cores, no collectives.
  - Conv as implicit GEMM: for each kernel tap (kh,kw), a matmul with
    lhsT = weight[ci, co_tile] (K=Cin=128 partitions, M=128) and
    rhs  = shifted x window [Cin=128, 9 rows x 54 cols = 486], accumulating
    all 9 taps into one PSUM bank. 2 cout tiles x 6 row groups x 4 images
    = 48 accumulation groups x 9 matmuls per core.
  - Inputs cast to bf16 on host (PE runs at full rate); accumulation fp32.
  - Output copied PSUM->SBUF as bf16 (halves output DMA bytes); bias add
    and f32 upcast happen on host.
  - Graduated PE warmup (small then full-width matmuls) keeps the PE busy
    through the input-DMA window so HAM is at full clock (k=8) when the
    real stream starts, with no idle gap that would trigger a downshift.
  - Critical input DMAs split so the first taps' weights and first rows of
    x arrive first; everything else is deferred behind early matmuls so
    the SDMA round-robin doesn't steal bandwidth from the critical path.
"""

import numpy as np
import ml_dtypes

import concourse.bass as bass
import concourse.mybir as mybir
from concourse import bacc
import concourse.tile as tile
from concourse.tile import add_dep_helper
from concourse.bass_utils import run_bass_kernel_spmd

N, CIN, H, W = 32, 128, 56, 56
COUT, KH, KW = 256, 3, 3
HO, WO = H - KH + 1, W - KW + 1  # 54, 54
NCORES = 8
NPER = N // NCORES  # 4 images per core
CTILES = COUT // 128  # 2
RG = 9                # output rows per PSUM group
NG = HO // RG         # 6 row groups
NPIX = RG * WO        # 486 <= 512 (one fp32 PSUM bank)

N_WU_SMALL = 33       # small warmup matmuls (32x64)
N_WU_BIG = 7          # full-width warmup matmuls (128x486)

BF16 = mybir.dt.bfloat16
F32 = mybir.dt.float32


def build_nc() -> bass.Bass:
    nc = bacc.Bacc(None)
    x_h = nc.dram_tensor("x", [NPER, CIN, H, W], BF16, kind="ExternalInput")
    w_h = nc.dram_tensor("w", [CIN, KH * KW * COUT], BF16, kind="ExternalInput")
    o_h = nc.dram_tensor("out", [NPER, COUT, HO, WO], BF16, kind="ExternalOutput")

    with tile.TileContext(nc) as tc:
        with (
            tc.tile_pool(name="wpool", bufs=1) as wpool,
            tc.tile_pool(name="xpool", bufs=4) as xpool,
            tc.tile_pool(name="opool", bufs=4) as opool,
            tc.tile_pool(name="psum", bufs=8, space="PSUM") as psum_pool,
        ):
            # PE warmup: matmuls on memset data with no DMA deps keep the PE
            # busy during the input-DMA window so HAM un-throttles to full
            # clock by the time the real matmuls begin. Graduated: small
            # matmuls first, then full-width ones so the power step of the
            # real stream doesn't trigger a downshift.
            # memset on DVE: keeps GpSimd entirely unused so its queue/boot
            # cost drops out of the NEFF.
            wu = wpool.tile([CIN, 640], BF16)
            nc.vector.memset(wu[:], 0)
            wupt = psum_pool.tile([32, 64], F32, tag="pt")
            warmups = []
            for _ in range(N_WU_SMALL):
                warmups.append(
                    nc.tensor.matmul(wupt[:], wu[:, :32], wu[:, :64], start=True, stop=True)
                )
            wupt2 = psum_pool.tile([128, NPIX], F32, tag="pt")
            for i in range(N_WU_BIG):
                warmups.append(
                    nc.tensor.matmul(
                        wupt2[:], wu[:, :128], wu[:, 128 : 128 + NPIX],
                        start=(i == 0), stop=(i == N_WU_BIG - 1),
                    )
                )

            # Input DMAs split across the two HWDGE rings (each is FIFO):
            # weights on sync (SP) in parallel with the first x0 chunk on
            # scalar (ACT). The weight stream is split per-tap-triplet so the
            # first real matmul only waits on taps 0-2; later chunks arrive
            # while the first groups run.
            wt = wpool.tile([CIN, KH * KW * COUT], BF16)
            nc.sync.dma_start(out=wt[:, : 3 * COUT], in_=w_h[:, : 3 * COUT])
            wB = nc.sync.dma_start(
                out=wt[:, 3 * COUT : 6 * COUT], in_=w_h[:, 3 * COUT : 6 * COUT]
            )
            wC = nc.sync.dma_start(out=wt[:, 6 * COUT :], in_=w_h[:, 6 * COUT :])

            xts = []
            for n in range(NPER):
                xt = xpool.tile([CIN, H, W], BF16, tag="xt", name=f"xt{n}")
                xts.append(xt)
            # group 0 needs x rows 0-10 only
            nc.scalar.dma_start(out=xts[0][:, 0:11, :], in_=x_h[0, :, 0:11, :])
            x0b = nc.scalar.dma_start(out=xts[0][:, 11:30, :], in_=x_h[0, :, 11:30, :])
            x0c = nc.scalar.dma_start(out=xts[0][:, 30:56, :], in_=x_h[0, :, 30:56, :])
            xdmas = [None]
            for n in range(1, NPER):
                xdmas.append(nc.sync.dma_start(out=xts[n][:], in_=x_h[n]))

            # Defer non-critical input DMAs behind warmup/early matmuls so
            # the SDMA round-robin doesn't steal bandwidth from the
            # transfers the first matmuls need. wB/wC stay undeferred: the
            # sync ring is FIFO, so they stream right behind wA.
            add_dep_helper(x0b.ins, warmups[30].ins, reason="defer x0b")
            deferred = {0: [x0c], 18: [xdmas[1]], 108: [xdmas[2]], 216: [xdmas[3]]}
            mm_idx = 0

            for n in range(NPER):
                xt = xts[n]
                for c in range(CTILES):
                    for g in range(NG):
                        pt = psum_pool.tile([128, RG, WO], F32, tag="pt")
                        for t in range(KH * KW):
                            kh, kw = divmod(t, KW)
                            lhsT = wt[:, t * COUT + c * 128 : t * COUT + c * 128 + 128]
                            rhs = xt[:, g * RG + kh : g * RG + kh + RG, kw : kw + WO]
                            mm = nc.tensor.matmul(
                                pt[:], lhsT, rhs,
                                start=(t == 0), stop=(t == KH * KW - 1),
                            )
                            for dma in deferred.get(mm_idx, ()):
                                add_dep_helper(dma.ins, mm.ins, reason="defer DMA")
                            mm_idx += 1
                        ot = opool.tile([128, RG, WO], BF16, tag="ot")
                        last = n == NPER - 1 and c == CTILES - 1 and g == NG - 1
                        co = c * 128
                        # Output DMAs ride the scalar (ACT) HWDGE ring so their
                        # sem waits never head-of-line block the input ring.
                        if not last:
                            nc.vector.tensor_copy(ot[:], pt[:])
                            nc.scalar.dma_start(
                                out=o_h[n, co : co + 128, g * RG : (g + 1) * RG, :],
                                in_=ot[:],
                            )
                        else:
                            # Split the final copy/DMA so the store pipeline
                            # drains sooner after the last matmul.
                            for eng, (ra, rb) in (
                                (nc.scalar, (0, 5)),
                                (nc.sync, (5, RG)),
                            ):
                                nc.vector.tensor_copy(ot[:, ra:rb, :], pt[:, ra:rb, :])
                                eng.dma_start(
                                    out=o_h[n, co : co + 128, g * RG + ra : g * RG + rb, :],
                                    in_=ot[:, ra:rb, :],
                                )
    nc.finalize()
    return nc


_NC_CACHE = None


def _get_nc():
    global _NC_CACHE
    if _NC_CACHE is None:
        _NC_CACHE = build_nc()
    return _NC_CACHE


def _prep_in_maps(x, weight):
    bf16 = ml_dtypes.bfloat16
    # [ci, kh, kw, co] layout so lhsT slices are [ci, co_tile]
    w_t = np.ascontiguousarray(
        weight.astype(np.float32).transpose(1, 2, 3, 0).reshape(CIN, KH * KW * COUT)
    ).astype(bf16)
    in_maps = []
    for i in range(NCORES):
        xs = np.ascontiguousarray(x[i * NPER : (i + 1) * NPER]).astype(bf16)
        in_maps.append({"x": xs, "w": w_t})
    return in_maps


def run(x, weight, bias, trace=False):
    nc = _get_nc()
    in_maps = _prep_in_maps(x, weight)
    res = run_bass_kernel_spmd(nc, in_maps, core_ids=list(range(NCORES)), trace=trace)
    out = np.concatenate([r["out"] for r in res.results], axis=0).astype(np.float32)
    bias = np.asarray(bias, dtype=np.float32)
    if np.any(bias):
        out += bias[None, :, None, None]
    return out, res


def kernel(x: np.ndarray, weight: np.ndarray, bias: np.ndarray) -> np.ndarray:
    out, _ = run(x, weight, bias, trace=False)
    return out.astype(np.float32)


# revision 22
# speedup vs baseline: 1.1082x; 1.0977x over previous
"""Conv2D (VALID, 3x3, NCHW) on 8 TRN2 NeuronCores via Bass/Tile.

Problem: x (32,128,56,56) f32, weight (256,128,3,3) f32, bias (256,) f32
         -> out (32,256,54,54) f32.

Strategy:
  - Data-parallel over batch: 4 images per core, 8 cores, no collectives.
  - Conv as implicit GEMM: per output tile (image, cout-half, 9-row group)
    one PSUM accumulation group of [128, 9x54=486] pixels.
  - Hybrid precision: taps 0-6 run as bf16 matmuls (K=Cin=128); taps 7+8
    are packed into ONE fp8(e4m3) DoubleRow matmul (K=2x128 via two
    k-tiles at 2x rate), cutting the group from 9 to 8 PE instructions.
    Everything is computed x32 (weights pre-scaled so fp8 stays in e4m3's
    normal range); the host divides by 32 after the f32 upcast. Measured
    rel err 1.79e-2 vs the 2e-2 gate (all-bf16 is 2.9e-3).
  - Graduated PE warmup (small then full-width matmuls) keeps the PE busy
    through the input-DMA window so HAM stays at full clock with no idle
    gap, and the real stream starts already ramped.
  - Critical input DMAs split so the first taps' weights and first rows of
    x arrive first; everything else is deferred behind early matmuls so
    the SDMA round-robin doesn't steal bandwidth from the critical path.
  - Output copied PSUM->SBUF as bf16 (halves output DMA bytes); bias add
    and f32 upcast happen on host.
"""

import numpy as np
import ml_dtypes

import concourse.bass as bass
import concourse.mybir as mybir
from concourse import bacc
import concourse.tile as tile
from concourse.tile import add_dep_helper
from concourse.bass_utils import run_bass_kernel_spmd

N, CIN, H, W = 32, 128, 56, 56
COUT, KH, KW = 256, 3, 3
HO, WO = H - KH + 1, W - KW + 1  # 54, 54
NCORES = 8
NPER = N // NCORES  # 4 images per core
CTILES = COUT // 128  # 2
RG = 9                # output rows per PSUM group
NG = HO // RG         # 6 row groups
NPIX = RG * WO        # 486 <= 512 (one fp32 PSUM bank)
BF_TAPS = (0, 1, 3, 4, 6, 7, 8)  # bf16 taps; taps 2=(0,2) & 5=(1,2) ride
NBF = len(BF_TAPS)               # one fp8 DoubleRow (k-tile stride = W)
SCALE = 32.0          # weights pre-scaled by this; host divides it out

N_WU_SMALL = 33       # small warmup matmuls (32x64)
N_WU_BIG = 7          # full-width warmup matmuls (128x486)

BF16 = mybir.dt.bfloat16
E4 = mybir.dt.float8e4
F32 = mybir.dt.float32
DR = mybir.MatmulPerfMode.DoubleRow


def build_nc() -> bass.Bass:
    nc = bacc.Bacc(None)
    x_h = nc.dram_tensor("x", [NPER, CIN, H, W], BF16, kind="ExternalInput")
    x8_h = nc.dram_tensor("x8", [NPER, CIN, H, W], E4, kind="ExternalInput")
    w_h = nc.dram_tensor("w", [CIN, NBF * COUT], BF16, kind="ExternalInput")
    w8_h = nc.dram_tensor("w8", [CIN, 2 * COUT], E4, kind="ExternalInput")
    o_h = nc.dram_tensor("out", [NPER, COUT, HO, WO], BF16, kind="ExternalOutput")

    with tile.TileContext(nc) as tc:
        with (
            tc.tile_pool(name="wpool", bufs=1) as wpool,
            tc.tile_pool(name="xpool", bufs=4) as xpool,
            tc.tile_pool(name="x8pool", bufs=4) as x8pool,
            tc.tile_pool(name="opool", bufs=4) as opool,
            tc.tile_pool(name="psum", bufs=8, space="PSUM") as psum_pool,
        ):
            # PE warmup: matmuls on memset data with no DMA deps keep the PE
            # busy during the input-DMA window so HAM is at full clock by the
            # time the real matmuls begin. Graduated: small matmuls first,
            # then full-width ones so the power step of the real stream
            # doesn't trigger a downshift. memset on DVE keeps GpSimd
            # entirely unused.
            wu = wpool.tile([CIN, 640], BF16)
            nc.vector.memset(wu[:], 0)
            wupt = psum_pool.tile([32, 64], F32, tag="pt")
            warmups = []
            for _ in range(N_WU_SMALL):
                warmups.append(
                    nc.tensor.matmul(wupt[:], wu[:, :32], wu[:, :64], start=True, stop=True)
                )
            wupt2 = psum_pool.tile([128, NPIX], F32, tag="pt")
            for i in range(N_WU_BIG):
                warmups.append(
                    nc.tensor.matmul(
                        wupt2[:], wu[:, :128], wu[:, 128 : 128 + NPIX],
                        start=(i == 0), stop=(i == N_WU_BIG - 1),
                    )
                )

            # Input DMAs split across the two HWDGE rings (each is FIFO):
            # weights on sync (SP) in parallel with the first x0 chunk on
            # scalar (ACT). The weight stream is split per-tap-triplet so the
            # first real matmul only waits on taps 0-2; later chunks arrive
            # while the first groups run.
            wt = wpool.tile([CIN, NBF * COUT], BF16)
            w8t = wpool.tile([CIN, 2, COUT], E4)
            nc.sync.dma_start(out=wt[:, : 3 * COUT], in_=w_h[:, : 3 * COUT])
            nc.sync.dma_start(out=wt[:, 3 * COUT :], in_=w_h[:, 3 * COUT :])
            nc.sync.dma_start(
                out=w8t[:], in_=w8_h.rearrange("p (i c) -> p i c", i=2)
            )

            xts, x8ts = [], []
            for n in range(NPER):
                xts.append(xpool.tile([CIN, H, W], BF16, tag="xt", name=f"xt{n}"))
                x8ts.append(x8pool.tile([CIN, H, W], E4, tag="x8t", name=f"x8t{n}"))
            # group 0 needs bf16 x rows 0-10 and fp8 x rows 0-9
            nc.scalar.dma_start(out=xts[0][:, 0:11, :], in_=x_h[0, :, 0:11, :])
            x80a = nc.scalar.dma_start(out=x8ts[0][:, 0:11, :], in_=x8_h[0, :, 0:11, :])
            x0b = nc.scalar.dma_start(out=xts[0][:, 11:30, :], in_=x_h[0, :, 11:30, :])
            x80b = nc.scalar.dma_start(out=x8ts[0][:, 11:56, :], in_=x8_h[0, :, 11:56, :])
            x0c = nc.scalar.dma_start(out=xts[0][:, 30:56, :], in_=x_h[0, :, 30:56, :])
            xdmas, x8dmas = [None], [None]
            for n in range(1, NPER):
                xdmas.append(nc.sync.dma_start(out=xts[n][:], in_=x_h[n]))
                x8dmas.append(nc.sync.dma_start(out=x8ts[n][:], in_=x8_h[n]))

            # Defer non-critical input DMAs behind warmup/early matmuls so
            # the SDMA round-robin doesn't steal bandwidth from the
            # transfers the first matmuls need.
            add_dep_helper(x0b.ins, warmups[30].ins, reason="defer x0b")
            add_dep_helper(x80b.ins, warmups[36].ins, reason="defer x80b")
            deferred = {
                0: [x0c],
                16: [xdmas[1]],
                24: [x8dmas[1]],
                96: [xdmas[2]],
                104: [x8dmas[2]],
                192: [xdmas[3]],
                200: [x8dmas[3]],
            }
            mm_idx = 0

            def dr_rhs(xt8, r0):
                # [128, 2, RG, WO] moving AP: k-tile i = tap (i, 2), i.e. a
                # one-row shift between the two k-tiles (stride W), the exact
                # AP shape validated in the DoubleRow numerics probe.
                v = xt8[:, r0 : r0 + RG, 2 : 2 + WO].unsqueeze(1)
                v.ap[:] = [
                    tuple(v.ap[0]), (W, 2), tuple(v.ap[2]), tuple(v.ap[3]),
                ]
                return v

            for n in range(NPER):
                xt, xt8 = xts[n], x8ts[n]
                for c in range(CTILES):
                    for g in range(NG):
                        pt = psum_pool.tile([128, RG, WO], F32, tag="pt")
                        for ti, t in enumerate(BF_TAPS):
                            kh, kw = divmod(t, KW)
                            lhsT = wt[:, ti * COUT + c * 128 : ti * COUT + c * 128 + 128]
                            rhs = xt[:, g * RG + kh : g * RG + kh + RG, kw : kw + WO]
                            mm = nc.tensor.matmul(
                                pt[:], lhsT, rhs, start=(ti == 0), stop=False
                            )
                            for dma in deferred.get(mm_idx, ()):
                                add_dep_helper(dma.ins, mm.ins, reason="defer DMA")
                            mm_idx += 1
                        # taps 2+5 in one fp8 DoubleRow matmul at 2x rate
                        mm = nc.tensor.matmul(
                            pt[:],
                            w8t[:, :, c * 128 : c * 128 + 128],
                            dr_rhs(xt8, g * RG),
                            start=False, stop=True, perf_mode=DR,
                        )
                        for dma in deferred.get(mm_idx, ()):
                            add_dep_helper(dma.ins, mm.ins, reason="defer DMA")
                        mm_idx += 1

                        ot = opool.tile([128, RG, WO], BF16, tag="ot")
                        last = n == NPER - 1 and c == CTILES - 1 and g == NG - 1
                        co = c * 128
                        # Output DMAs ride the scalar (ACT) HWDGE ring so their
                        # sem waits never head-of-line block the input ring.
                        if not last:
                            nc.vector.tensor_copy(ot[:], pt[:])
                            nc.scalar.dma_start(
                                out=o_h[n, co : co + 128, g * RG : (g + 1) * RG, :],
                                in_=ot[:],
                            )
                        else:
                            # Split the final copy/DMA so the store pipeline
                            # drains sooner after the last matmul.
                            for eng, (ra, rb) in (
                                (nc.scalar, (0, 5)),
                                (nc.sync, (5, RG)),
                            ):
                                nc.vector.tensor_copy(ot[:, ra:rb, :], pt[:, ra:rb, :])
                                eng.dma_start(
                                    out=o_h[n, co : co + 128, g * RG + ra : g * RG + rb, :],
                                    in_=ot[:, ra:rb, :],
                                )
    nc.finalize()
    return nc


_NC_CACHE = None


def _get_nc():
    global _NC_CACHE
    if _NC_CACHE is None:
        _NC_CACHE = build_nc()
    return _NC_CACHE


def _prep_in_maps(x, weight):
    bf16 = ml_dtypes.bfloat16
    e4 = ml_dtypes.float8_e4m3
    # [ci, kh*kw, co] layout so lhsT slices are [ci, co_tile]
    w32 = (weight.astype(np.float32) * SCALE).transpose(1, 2, 3, 0).reshape(
        CIN, KH * KW, COUT
    )
    w_t = np.ascontiguousarray(
        w32[:, list(BF_TAPS)].reshape(CIN, NBF * COUT)
    ).astype(bf16)
    w8_t = np.ascontiguousarray(w32[:, [2, 5]].reshape(CIN, 2 * COUT)).astype(e4)
    in_maps = []
    for i in range(NCORES):
        xs = np.ascontiguousarray(x[i * NPER : (i + 1) * NPER])
        in_maps.append({
            "x": xs.astype(bf16),
            "x8": xs.astype(e4),
            "w": w_t,
            "w8": w8_t,
        })
    return in_maps


def run(x, weight, bias, trace=False):
    nc = _get_nc()
    in_maps = _prep_in_maps(x, weight)
    res = run_bass_kernel_spmd(nc, in_maps, core_ids=list(range(NCORES)), trace=trace)
    out = np.concatenate([r["out"] for r in res.results], axis=0).astype(np.float32)
    out *= 1.0 / SCALE
    bias = np.asarray(bias, dtype=np.float32)
    if np.any(bias):
        out += bias[None, :, None, None]
    return out, res


def kernel(x: np.ndarray, weight: np.ndarray, bias: np.ndarray) -> np.ndarray:
    out, _ = run(x, weight, bias, trace=False)
    return out.astype(np.float32)
